# revision 2
# baseline (speedup 1.0000x reference)
"""Multi-head causal attention with RoPE on 8 TRN2 NeuronCores.

Tensor-parallel over heads: core c computes heads (2c, 2c+1).
  Phase 1: Q^T,K^T (with RoPE) and V projections from pre-transposed x.
  Phase 2: causal attention per (batch, head) in transposed orientation
           (scores^T = K^T_blk^T @ Q^T), softmax without max-subtraction
           (scores are O(1) here), softmax denominators via ones-matmul.
  Phase 3: AllGather of per-core context^T, then out^T = Wo_cols^T @ ctx^T.
Host does layout prep (x transpose, RoPE tables, causal mask tiles) and
final unshard (concat output column blocks).

All TensorEngine operands use float32r (~2^-13 relative rounding, 4x faster
than fp32 matmul); accumulation is fp32 in PSUM.
"""
import numpy as np

import concourse.bass as bass  # noqa: F401  (engine namespaces live on nc)
import concourse.mybir as mybir
import concourse.tile as tile
from concourse import bacc
from concourse import bass_utils

B, T, DM, H, D = 2, 2048, 2048, 16, 128
NCORES = 8
HPC = H // NCORES        # heads per core
DLOC = HPC * D           # local head width (256)
BT = B * T               # 4096 token rows
P = 128
TCH = 512                # free-dim chunk
NKB = DM // P            # 16 contraction blocks
NTCH = BT // TCH         # 8 token chunks
NTB = T // P             # 16 token blocks per batch
SCALE = 1.0 / float(np.sqrt(D))
F32 = mybir.dt.float32
F32R = mybir.dt.float32r
MUL = mybir.AluOpType.mult
ADD = mybir.AluOpType.add

_nc_cache = None


def _build():
    nc = bacc.Bacc("TRN2", target_bir_lowering=False, debug=False,
                   num_devices=NCORES)
    xt = nc.dram_tensor("xt", [DM, BT], F32R, kind="ExternalInput")
    wq = nc.dram_tensor("wq", [DM, DLOC], F32R, kind="ExternalInput")
    wk = nc.dram_tensor("wk", [DM, DLOC], F32R, kind="ExternalInput")
    wv = nc.dram_tensor("wv", [DM, DLOC], F32R, kind="ExternalInput")
    wo = nc.dram_tensor("wo", [DM, DLOC], F32R, kind="ExternalInput")
    cf = nc.dram_tensor("cf", [P, T], F32, kind="ExternalInput")
    sf = nc.dram_tensor("sf", [P, T], F32, kind="ExternalInput")
    cm = nc.dram_tensor("cm", [P, 4 * TCH], F32, kind="ExternalInput")
    onec = nc.dram_tensor("onec", [P, 1], F32R, kind="ExternalInput")
    oner = nc.dram_tensor("oner", [1, P], F32R, kind="ExternalInput")
    outT = nc.dram_tensor("out", [DLOC, BT], F32, kind="ExternalOutput")

    with tile.TileContext(nc) as tc:
        with tc.tile_pool(name="dram", bufs=1, space="DRAM") as dpool, \
             tc.tile_pool(name="const", bufs=1) as cpool:
            qT_d = dpool.tile([DLOC, BT], F32R)
            kT_d = dpool.tile([DLOC, BT], F32R)
            v_d = dpool.tile([BT, DLOC], F32R)
            ctxT_d = dpool.tile([DLOC, BT], F32R)
            gath_d = dpool.tile([DM, BT], F32R, addr_space="Shared")

            cf_s = cpool.tile([P, T], F32)
            sf_s = cpool.tile([P, T], F32)
            cm_s = cpool.tile([P, 4 * TCH], F32)
            onec_s = cpool.tile([P, 1], F32R)
            oner_s = cpool.tile([1, P], F32R)
            nc.sync.dma_start(cf_s[:], cf.ap())
            nc.sync.dma_start(sf_s[:], sf.ap())
            nc.sync.dma_start(cm_s[:], cm.ap())
            nc.sync.dma_start(onec_s[:], onec.ap())
            nc.sync.dma_start(oner_s[:], oner.ap())

            # ---------------- Phase 1: projections + RoPE ----------------
            with tc.tile_pool(name="p1w", bufs=1) as wpool, \
                 tc.tile_pool(name="p1", bufs=2) as pool, \
                 tc.tile_pool(name="ps1", bufs=2, space="PSUM") as ps1:
                wq_s = wpool.tile([P, NKB, DLOC], F32R)
                wk_s = wpool.tile([P, NKB, DLOC], F32R)
                wv_s = wpool.tile([P, NKB, DLOC], F32R)
                nc.sync.dma_start(wq_s[:], wq.ap().rearrange("(kb p) m -> p kb m", p=P))
                nc.sync.dma_start(wk_s[:], wk.ap().rearrange("(kb p) m -> p kb m", p=P))
                nc.sync.dma_start(wv_s[:], wv.ap().rearrange("(kb p) m -> p kb m", p=P))

                for i in range(NTCH):
                    xt_t = pool.tile([P, NKB, TCH], F32R, tag="xt")
                    nc.sync.dma_start(
                        xt_t[:],
                        xt.ap()[:, i * TCH:(i + 1) * TCH]
                        .rearrange("(kb p) n -> p kb n", p=P))
                    bcol = (i % (T // TCH)) * TCH  # column offset in rope tables
                    cs = cf_s[:, bcol:bcol + TCH]
                    sn = sf_s[:, bcol:bcol + TCH]
                    for w_s, dst in ((wq_s, qT_d), (wk_s, kT_d)):
                        for m in range(HPC):
                            ps = ps1.tile([P, TCH], F32, tag="qk")
                            for kb in range(NKB):
                                nc.tensor.matmul(
                                    ps[:], w_s[:, kb, m * P:(m + 1) * P],
                                    xt_t[:, kb],
                                    start=(kb == 0), stop=(kb == NKB - 1))
                            # RoPE: rq = q*cos_full + rot(q)*sin_signed
                            tmp = pool.tile([P, TCH], F32, tag="tmp")
                            tmp2 = pool.tile([P, TCH], F32, tag="tmp2")
                            rq = pool.tile([P, TCH], F32R, tag="rq")
                            nc.vector.tensor_tensor(tmp[0:64], ps[64:128], sn[0:64], MUL)
                            nc.vector.tensor_tensor(tmp[64:128], ps[0:64], sn[64:128], MUL)
                            nc.vector.tensor_tensor(tmp2[:], ps[:], cs, MUL)
                            nc.vector.tensor_tensor(rq[:], tmp2[:], tmp[:], ADD)
                            nc.sync.dma_start(
                                dst[m * P:(m + 1) * P, i * TCH:(i + 1) * TCH],
                                rq[:])
                    for tb in range(TCH // P):
                        psv = ps1.tile([P, DLOC], F32, tag="v")
                        for kb in range(NKB):
                            nc.tensor.matmul(
                                psv[:], xt_t[:, kb, tb * P:(tb + 1) * P],
                                wv_s[:, kb],
                                start=(kb == 0), stop=(kb == NKB - 1))
                        vsb = pool.tile([P, DLOC], F32R, tag="vsb")
                        nc.vector.tensor_copy(vsb[:], psv[:])
                        r0 = i * TCH + tb * P
                        nc.sync.dma_start(v_d[r0:r0 + P, :], vsb[:])

            # ---------------- Phase 2: causal attention ----------------
            with tc.tile_pool(name="p2", bufs=2) as pool2, \
                 tc.tile_pool(name="p2t", bufs=4) as ppool, \
                 tc.tile_pool(name="ps_s", bufs=2, space="PSUM") as ps_sp, \
                 tc.tile_pool(name="ps_acc", bufs=2, space="PSUM") as ps_accp, \
                 tc.tile_pool(name="ps_misc", bufs=1, space="PSUM") as ps_mp:
                for b in range(B):
                    for hl in range(HPC):
                        kT_s = pool2.tile([P, T], F32R, tag="kT")
                        nc.sync.dma_start(
                            kT_s[:], kT_d[hl * P:(hl + 1) * P, b * T:(b + 1) * T])
                        v_s = pool2.tile([P, NTB, D], F32R, tag="v")
                        nc.sync.dma_start(
                            v_s[:],
                            v_d[b * T:(b + 1) * T, hl * D:(hl + 1) * D]
                            .rearrange("(j p) d -> p j d", p=P))
                        for cq in range(T // TCH):
                            qT_s = pool2.tile([P, TCH], F32R, tag="qT")
                            c0 = b * T + cq * TCH
                            nc.sync.dma_start(
                                qT_s[:], qT_d[hl * P:(hl + 1) * P, c0:c0 + TCH])
                            nblk = 4 * cq + 4
                            ps_ctx = ps_accp.tile([P, TCH], F32, tag="ctx")
                            ps_sum = ps_mp.tile([1, TCH], F32, tag="sum")
                            for j in range(nblk):
                                ps_sc = ps_sp.tile([P, TCH], F32, tag="s")
                                nc.tensor.matmul(
                                    ps_sc[:], kT_s[:, j * P:(j + 1) * P], qT_s[:],
                                    start=True, stop=True)
                                pT = ppool.tile([P, TCH], F32R, tag="pT")
                                nc.scalar.activation(
                                    pT[:], ps_sc[:],
                                    mybir.ActivationFunctionType.Exp, scale=SCALE)
                                vmask = j - 4 * cq
                                if vmask >= 0:
                                    nc.vector.tensor_tensor(
                                        pT[:], pT[:],
                                        cm_s[:, vmask * TCH:(vmask + 1) * TCH], MUL)
                                nc.tensor.matmul(
                                    ps_ctx[:], v_s[:, j], pT[:],
                                    start=(j == 0), stop=(j == nblk - 1))
                                nc.tensor.matmul(
                                    ps_sum[:], onec_s[:], pT[:],
                                    start=(j == 0), stop=(j == nblk - 1))
                            rs = pool2.tile([1, TCH], F32R, tag="rs")
                            with nc.allow_low_precision(reason="f32r rounding of softmax denom"):
                                nc.vector.reciprocal(rs[:], ps_sum[:])
                            ps_bc = ps_mp.tile([P, TCH], F32, tag="bc")
                            nc.tensor.matmul(ps_bc[:], oner_s[:], rs[:],
                                             start=True, stop=True)
                            bc_s = pool2.tile([P, TCH], F32, tag="bc_s")
                            nc.vector.tensor_copy(bc_s[:], ps_bc[:])
                            ctx_s = pool2.tile([P, TCH], F32R, tag="ctx")
                            nc.vector.tensor_tensor(ctx_s[:], ps_ctx[:], bc_s[:], MUL)
                            nc.sync.dma_start(
                                ctxT_d[hl * P:(hl + 1) * P, c0:c0 + TCH], ctx_s[:])

            # ---------------- Phase 3: AllGather + output proj ----------------
            with tc.tile_pool(name="p3w", bufs=1) as wpool3, \
                 tc.tile_pool(name="p3", bufs=2) as pool3, \
                 tc.tile_pool(name="ps3", bufs=2, space="PSUM") as ps3:
                nc.gpsimd.collective_compute(
                    "AllGather", mybir.AluOpType.bypass,
                    replica_groups=[list(range(NCORES))],
                    ins=[ctxT_d[:].opt()],
                    outs=[gath_d[:].opt()])
                wo_s = wpool3.tile([P, NKB, DLOC], F32R)
                nc.sync.dma_start(wo_s[:], wo.ap().rearrange("(kb p) m -> p kb m", p=P))
                for i in range(NTCH):
                    g_t = pool3.tile([P, NKB, TCH], F32R, tag="g")
                    nc.sync.dma_start(
                        g_t[:],
                        gath_d[:, i * TCH:(i + 1) * TCH]
                        .rearrange("(kb p) n -> p kb n", p=P))
                    for m in range(HPC):
                        pso = ps3.tile([P, TCH], F32, tag="o")
                        for kb in range(NKB):
                            nc.tensor.matmul(
                                pso[:], wo_s[:, kb, m * P:(m + 1) * P], g_t[:, kb],
                                start=(kb == 0), stop=(kb == NKB - 1))
                        o_s = pool3.tile([P, TCH], F32, tag="o_s")
                        nc.vector.tensor_copy(o_s[:], pso[:])
                        nc.sync.dma_start(
                            outT.ap()[m * P:(m + 1) * P, i * TCH:(i + 1) * TCH],
                            o_s[:])

    nc.compile()
    return nc


def _prep_inputs(x, cos, sin, Wq, Wk, Wv, Wo):
    x = np.asarray(x, dtype=np.float32)
    cos = np.asarray(cos, dtype=np.float32)
    sin = np.asarray(sin, dtype=np.float32)
    xt = np.ascontiguousarray(x.reshape(BT, DM).T)
    cf = np.empty((P, T), np.float32)
    cf[:64] = cos.T
    cf[64:] = cos.T
    sf = np.empty((P, T), np.float32)
    sf[:64] = -sin.T
    sf[64:] = sin.T
    q = np.arange(TCH, dtype=np.int64)[None, :]
    r = np.arange(P, dtype=np.int64)[:, None]
    cm = np.concatenate(
        [(q >= v * P + r).astype(np.float32) for v in range(TCH // P)], axis=1)
    onec = np.ones((P, 1), np.float32)
    oner = np.ones((1, P), np.float32)
    in_maps = []
    for c in range(NCORES):
        sl = slice(c * DLOC, (c + 1) * DLOC)
        in_maps.append({
            "xt": xt, "cf": cf, "sf": sf, "cm": cm,
            "onec": onec, "oner": oner,
            "wq": np.ascontiguousarray(np.asarray(Wq, np.float32)[:, sl]),
            "wk": np.ascontiguousarray(np.asarray(Wk, np.float32)[:, sl]),
            "wv": np.ascontiguousarray(np.asarray(Wv, np.float32)[:, sl]),
            "wo": np.ascontiguousarray(np.asarray(Wo, np.float32)[:, sl]),
        })
    return in_maps


def run(x, mask, cos, sin, Wq, Wk, Wv, Wo, trace=False):
    global _nc_cache
    if _nc_cache is None:
        _nc_cache = _build()
    in_maps = _prep_inputs(x, cos, sin, Wq, Wk, Wv, Wo)
    res = bass_utils.run_bass_kernel_spmd(
        _nc_cache, in_maps, core_ids=list(range(NCORES)), trace=trace)
    out = np.empty((BT, DM), np.float32)
    for c in range(NCORES):
        out[:, c * DLOC:(c + 1) * DLOC] = res.results[c]["out"].T
    return out.reshape(B, T, DM), res


def kernel(x, mask, cos, sin, Wq, Wk, Wv, Wo):
    out, _ = run(x, mask, cos, sin, Wq, Wk, Wv, Wo, trace=False)
    return out


# revision 3
# speedup vs baseline: 1.1965x; 1.1965x over previous
"""Multi-head causal attention with RoPE on 8 TRN2 NeuronCores.

Tensor-parallel over heads: core c computes heads (2c, 2c+1).
  Phase 1: Q^T,K^T (with RoPE) and V projections from pre-transposed x.
  Phase 2: causal attention per (batch, head) in transposed orientation
           (scores^T = K^T_blk^T @ Q^T), softmax without max-subtraction
           (scores are O(1) here), softmax denominators via ones-matmul.
  Phase 3: AllGather of per-core context^T, then out^T = Wo_cols^T @ ctx^T.
Host does layout prep (x transpose, RoPE tables, causal mask tiles) and
final unshard (concat output column blocks).

All TensorEngine operands use float32r (~2^-13 relative rounding, 4x faster
than fp32 matmul); accumulation is fp32 in PSUM.
"""
import ml_dtypes
import numpy as np

import concourse.bass as bass  # noqa: F401  (engine namespaces live on nc)
import concourse.mybir as mybir
import concourse.tile as tile
from concourse import bacc
from concourse import bass_utils

B, T, DM, H, D = 2, 2048, 2048, 16, 128
NCORES = 8
HPC = H // NCORES        # heads per core
DLOC = HPC * D           # local head width (256)
BT = B * T               # 4096 token rows
P = 128
TCH = 512                # free-dim chunk
NKB = DM // P            # 16 contraction blocks
NTCH = BT // TCH         # 8 token chunks
NTB = T // P             # 16 token blocks per batch
SCALE = 1.0 / float(np.sqrt(D))
F32 = mybir.dt.float32
F32R = mybir.dt.float32r
BF16 = mybir.dt.bfloat16
MUL = mybir.AluOpType.mult
ADD = mybir.AluOpType.add

_nc_cache = None


def _build():
    nc = bacc.Bacc("TRN2", target_bir_lowering=False, debug=False,
                   num_devices=NCORES)
    xt = nc.dram_tensor("xt", [DM, BT], F32R, kind="ExternalInput")
    wq = nc.dram_tensor("wq", [DM, DLOC], F32R, kind="ExternalInput")
    wk = nc.dram_tensor("wk", [DM, DLOC], F32R, kind="ExternalInput")
    wv = nc.dram_tensor("wv", [DM, DLOC], F32R, kind="ExternalInput")
    wo = nc.dram_tensor("wo", [DM, DLOC], BF16, kind="ExternalInput")
    cf = nc.dram_tensor("cf", [P, T], F32, kind="ExternalInput")
    sf = nc.dram_tensor("sf", [P, T], F32, kind="ExternalInput")
    cm = nc.dram_tensor("cm", [P, 4 * TCH], F32, kind="ExternalInput")
    onec = nc.dram_tensor("onec", [P, 1], F32R, kind="ExternalInput")
    oner = nc.dram_tensor("oner", [1, P], F32R, kind="ExternalInput")
    outT = nc.dram_tensor("out", [DLOC, BT], F32, kind="ExternalOutput")

    with tile.TileContext(nc) as tc:
        with tc.tile_pool(name="dram", bufs=1, space="DRAM") as dpool, \
             tc.tile_pool(name="const", bufs=1) as cpool:
            qT_d = dpool.tile([DLOC, BT], F32R)
            kT_d = dpool.tile([DLOC, BT], F32R)
            v_d = dpool.tile([BT, DLOC], F32R)
            ctxT_d = [dpool.tile([DLOC, T], BF16, name=f"ctxT{b}") for b in range(B)]
            gath_d = [dpool.tile([DM, T], BF16, addr_space="Shared", name=f"gath{b}")
                      for b in range(B)]

            cf_s = cpool.tile([P, T], F32)
            sf_s = cpool.tile([P, T], F32)
            cm_s = cpool.tile([P, 4 * TCH], F32)
            onec_s = cpool.tile([P, 1], F32R)
            oner_s = cpool.tile([1, P], F32R)
            nc.sync.dma_start(cf_s[:], cf.ap())
            nc.sync.dma_start(sf_s[:], sf.ap())
            nc.sync.dma_start(cm_s[:], cm.ap())
            nc.sync.dma_start(onec_s[:], onec.ap())
            nc.sync.dma_start(oner_s[:], oner.ap())

            # ---------------- Phase 1: projections + RoPE ----------------
            with tc.tile_pool(name="p1w", bufs=1) as wpool, \
                 tc.tile_pool(name="p1", bufs=2) as pool, \
                 tc.tile_pool(name="ps1", bufs=2, space="PSUM") as ps1:
                wq_s = wpool.tile([P, NKB, DLOC], F32R)
                wk_s = wpool.tile([P, NKB, DLOC], F32R)
                wv_s = wpool.tile([P, NKB, DLOC], F32R)
                nc.sync.dma_start(wq_s[:], wq.ap().rearrange("(kb p) m -> p kb m", p=P))
                nc.sync.dma_start(wk_s[:], wk.ap().rearrange("(kb p) m -> p kb m", p=P))
                nc.sync.dma_start(wv_s[:], wv.ap().rearrange("(kb p) m -> p kb m", p=P))

                for i in range(NTCH):
                    xt_t = pool.tile([P, NKB, TCH], F32R, tag="xt")
                    nc.sync.dma_start(
                        xt_t[:],
                        xt.ap()[:, i * TCH:(i + 1) * TCH]
                        .rearrange("(kb p) n -> p kb n", p=P))
                    bcol = (i % (T // TCH)) * TCH  # column offset in rope tables
                    cs = cf_s[:, bcol:bcol + TCH]
                    sn = sf_s[:, bcol:bcol + TCH]
                    for w_s, dst in ((wq_s, qT_d), (wk_s, kT_d)):
                        for m in range(HPC):
                            ps = ps1.tile([P, TCH], F32, tag="qk")
                            for kb in range(NKB):
                                nc.tensor.matmul(
                                    ps[:], w_s[:, kb, m * P:(m + 1) * P],
                                    xt_t[:, kb],
                                    start=(kb == 0), stop=(kb == NKB - 1))
                            # RoPE: rq = q*cos_full + rot(q)*sin_signed
                            tmp = pool.tile([P, TCH], F32, tag="tmp")
                            tmp2 = pool.tile([P, TCH], F32, tag="tmp2")
                            rq = pool.tile([P, TCH], F32R, tag="rq")
                            nc.vector.tensor_tensor(tmp[0:64], ps[64:128], sn[0:64], MUL)
                            nc.vector.tensor_tensor(tmp[64:128], ps[0:64], sn[64:128], MUL)
                            nc.vector.tensor_tensor(tmp2[:], ps[:], cs, MUL)
                            nc.vector.tensor_tensor(rq[:], tmp2[:], tmp[:], ADD)
                            nc.sync.dma_start(
                                dst[m * P:(m + 1) * P, i * TCH:(i + 1) * TCH],
                                rq[:])
                    for tb in range(TCH // P):
                        psv = ps1.tile([P, DLOC], F32, tag="v")
                        for kb in range(NKB):
                            nc.tensor.matmul(
                                psv[:], xt_t[:, kb, tb * P:(tb + 1) * P],
                                wv_s[:, kb],
                                start=(kb == 0), stop=(kb == NKB - 1))
                        vsb = pool.tile([P, DLOC], F32R, tag="vsb")
                        nc.vector.tensor_copy(vsb[:], psv[:])
                        r0 = i * TCH + tb * P
                        nc.sync.dma_start(v_d[r0:r0 + P, :], vsb[:])

            # ---------------- Phase 2: causal attention ----------------
            with tc.tile_pool(name="p2", bufs=2) as pool2, \
                 tc.tile_pool(name="p2t", bufs=6) as ppool, \
                 tc.tile_pool(name="ps_s", bufs=2, space="PSUM") as ps_sp, \
                 tc.tile_pool(name="ps_acc", bufs=2, space="PSUM") as ps_accp, \
                 tc.tile_pool(name="ps_misc", bufs=2, space="PSUM") as ps_mp:
                for b in range(B):
                    for hl in range(HPC):
                        kT_s = pool2.tile([P, T], F32R, tag="kT")
                        nc.sync.dma_start(
                            kT_s[:], kT_d[hl * P:(hl + 1) * P, b * T:(b + 1) * T])
                        v_s = pool2.tile([P, NTB, D], F32R, tag="v")
                        nc.sync.dma_start(
                            v_s[:],
                            v_d[b * T:(b + 1) * T, hl * D:(hl + 1) * D]
                            .rearrange("(j p) d -> p j d", p=P))
                        for cq in range(T // TCH):
                            qT_s = pool2.tile([P, TCH], F32R, tag="qT")
                            c0 = b * T + cq * TCH
                            nc.sync.dma_start(
                                qT_s[:], qT_d[hl * P:(hl + 1) * P, c0:c0 + TCH])
                            nblk = 4 * cq + 4
                            ps_ctx = ps_accp.tile([P, TCH], F32, tag="ctx")
                            ps_sum = ps_mp.tile([1, TCH], F32, tag="sum")
                            for j in range(nblk):
                                ps_sc = ps_sp.tile([P, TCH], F32, tag="s")
                                nc.tensor.matmul(
                                    ps_sc[:], kT_s[:, j * P:(j + 1) * P], qT_s[:],
                                    start=True, stop=True)
                                pT = ppool.tile([P, TCH], F32R, tag="pT")
                                nc.scalar.activation(
                                    pT[:], ps_sc[:],
                                    mybir.ActivationFunctionType.Exp, scale=SCALE)
                                vmask = j - 4 * cq
                                if vmask >= 0:
                                    nc.vector.tensor_tensor(
                                        pT[:], pT[:],
                                        cm_s[:, vmask * TCH:(vmask + 1) * TCH], MUL)
                                nc.tensor.matmul(
                                    ps_ctx[:], v_s[:, j], pT[:],
                                    start=(j == 0), stop=(j == nblk - 1))
                                nc.tensor.matmul(
                                    ps_sum[:], onec_s[:], pT[:],
                                    start=(j == 0), stop=(j == nblk - 1))
                            rs = pool2.tile([1, TCH], F32R, tag="rs")
                            with nc.allow_low_precision(reason="f32r rounding of softmax denom"):
                                nc.vector.reciprocal(rs[:], ps_sum[:])
                            ps_bc = ps_mp.tile([P, TCH], F32, tag="bc")
                            nc.tensor.matmul(ps_bc[:], oner_s[:], rs[:],
                                             start=True, stop=True)
                            bc_s = pool2.tile([P, TCH], F32, tag="bc_s")
                            nc.vector.tensor_copy(bc_s[:], ps_bc[:])
                            ctx_s = pool2.tile([P, TCH], BF16, tag="ctx")
                            nc.vector.tensor_tensor(ctx_s[:], ps_ctx[:], bc_s[:], MUL)
                            nc.sync.dma_start(
                                ctxT_d[b][hl * P:(hl + 1) * P,
                                          cq * TCH:(cq + 1) * TCH], ctx_s[:])
                    nc.gpsimd.collective_compute(
                        "AllGather", mybir.AluOpType.bypass,
                        replica_groups=[list(range(NCORES))],
                        ins=[ctxT_d[b][:].opt()],
                        outs=[gath_d[b][:].opt()])

            # ---------------- Phase 3: AllGather + output proj ----------------
            with tc.tile_pool(name="p3w", bufs=1) as wpool3, \
                 tc.tile_pool(name="p3", bufs=2) as pool3, \
                 tc.tile_pool(name="ps3", bufs=2, space="PSUM") as ps3:
                wo_s = wpool3.tile([P, NKB, DLOC], BF16)
                nc.sync.dma_start(wo_s[:], wo.ap().rearrange("(kb p) m -> p kb m", p=P))
                for i in range(NTCH):
                    bb = i // (T // TCH)
                    g_t = pool3.tile([P, NKB, TCH], BF16, tag="g")
                    nc.sync.dma_start(
                        g_t[:],
                        gath_d[bb][:, (i % (T // TCH)) * TCH:(i % (T // TCH) + 1) * TCH]
                        .rearrange("(kb p) n -> p kb n", p=P))
                    for m in range(HPC):
                        pso = ps3.tile([P, TCH], F32, tag="o")
                        for kb in range(NKB):
                            nc.tensor.matmul(
                                pso[:], wo_s[:, kb, m * P:(m + 1) * P], g_t[:, kb],
                                start=(kb == 0), stop=(kb == NKB - 1))
                        o_s = pool3.tile([P, TCH], F32, tag="o_s")
                        nc.vector.tensor_copy(o_s[:], pso[:])
                        nc.sync.dma_start(
                            outT.ap()[m * P:(m + 1) * P, i * TCH:(i + 1) * TCH],
                            o_s[:])

    nc.compile()
    return nc


def _prep_inputs(x, cos, sin, Wq, Wk, Wv, Wo):
    x = np.asarray(x, dtype=np.float32)
    cos = np.asarray(cos, dtype=np.float32)
    sin = np.asarray(sin, dtype=np.float32)
    xt = np.ascontiguousarray(x.reshape(BT, DM).T)
    cf = np.empty((P, T), np.float32)
    cf[:64] = cos.T
    cf[64:] = cos.T
    sf = np.empty((P, T), np.float32)
    sf[:64] = -sin.T
    sf[64:] = sin.T
    q = np.arange(TCH, dtype=np.int64)[None, :]
    r = np.arange(P, dtype=np.int64)[:, None]
    cm = np.concatenate(
        [(q >= v * P + r).astype(np.float32) for v in range(TCH // P)], axis=1)
    onec = np.ones((P, 1), np.float32)
    oner = np.ones((1, P), np.float32)
    in_maps = []
    for c in range(NCORES):
        sl = slice(c * DLOC, (c + 1) * DLOC)
        in_maps.append({
            "xt": xt, "cf": cf, "sf": sf, "cm": cm,
            "onec": onec, "oner": oner,
            "wq": np.ascontiguousarray(np.asarray(Wq, np.float32)[:, sl]),
            "wk": np.ascontiguousarray(np.asarray(Wk, np.float32)[:, sl]),
            "wv": np.ascontiguousarray(np.asarray(Wv, np.float32)[:, sl]),
            "wo": np.ascontiguousarray(
                np.asarray(Wo, np.float32)[:, sl]).astype(ml_dtypes.bfloat16),
        })
    return in_maps


def run(x, mask, cos, sin, Wq, Wk, Wv, Wo, trace=False):
    global _nc_cache
    if _nc_cache is None:
        _nc_cache = _build()
    in_maps = _prep_inputs(x, cos, sin, Wq, Wk, Wv, Wo)
    res = bass_utils.run_bass_kernel_spmd(
        _nc_cache, in_maps, core_ids=list(range(NCORES)), trace=trace)
    out = np.empty((BT, DM), np.float32)
    for c in range(NCORES):
        out[:, c * DLOC:(c + 1) * DLOC] = res.results[c]["out"].T
    return out.reshape(B, T, DM), res


def kernel(x, mask, cos, sin, Wq, Wk, Wv, Wo):
    out, _ = run(x, mask, cos, sin, Wq, Wk, Wv, Wo, trace=False)
    return out
